# revision 7
# baseline (speedup 1.0000x reference)
"""BitNet linear layer (b1.58-style) on 8 Trainium2 NeuronCores.

Computes: scale = 1e-4 + mean(|W|); q = clip(round(W/scale), -1, 1);
          out = scale * (x @ q.T)
for x [4, 2048, 2048] f32 and W [8192, 2048] f32.

Sharding: tensor-parallel over out_features. Each core gets the full x
(replicated) and a 1024-row shard of the ternary q; cores run fully
independently and the host concatenates the per-core [8192, 1024]
output slices along the feature axis.

The elementwise prep is done once on the host (it is ~0.1% of the FLOPs
and would otherwise be redundantly recomputed per core): the exact
global scale and ternary q (bit-identical rounding vs the reference),
the f32->bf16 casts, and the transposes into SBUF-ready layouts.
`scale` is folded into the bf16 x cast, which is free in accuracy terms
(a single bf16 rounding either way), so the device applies no scale at
all. Remaining error is just the bf16 rounding of x (~2.3e-3).

The device is then a pure gap-free bf16 matmul at the PE roofline
(2048 matmuls of N=512 at ~216 ns cadence ~= 443 us):

  - xdev [8192, 2048] bf16 (replicated): row mt*128+p, col ko*128+m
    holds scale*x[token mt*128+m, k = ko*128+p] -- i.e. 64 m-tiles,
    each a [128k x (16ko x 128m)] stationary-operand block, 4 KiB
    contiguous per partition. One 512 KiB DMA per m-tile on the
    scalar queue, prefetched ~12 tiles deep.
  - qdev [128, 16384] bf16 (per-core shard): col ko*1024+n holds
    q[n-th row of shard, ko*128+p]. Loaded as 16 per-ko slices,
    even ko on the sync queue, odd ko on gpsimd, so the first
    matmul only waits for slice 0 and the rest land under compute.
  - Main loop over 64 m-tiles: x tile is the stationary operand
    (LDWEIGHTS hides under the moving stream), q is the moving
    operand; 16 k-steps of two 512-col accumulating matmuls into a
    psum bank pair (8 banks -> 4 m-tiles in flight). DVE drains
    psum -> f32 out tile; out rows stored in natural [M, N-shard]
    orientation on the sync queue.
"""

import sys

sys.path.insert(0, "/opt/trn_rl_repo")

import numpy as np
import ml_dtypes

import concourse.bass as bass
import concourse.tile as tile
from concourse import bacc, mybir
from concourse.bass_utils import run_bass_kernel_spmd

F32 = mybir.dt.float32
BF16 = mybir.dt.bfloat16
FP8 = mybir.dt.float8e4
BF16_NP = ml_dtypes.bfloat16
FP8_NP = ml_dtypes.float8_e4m3

NCORES = 8
M = 8192          # tokens (4*2048)
K = 2048          # in_features
N_FULL = 8192     # out_features
NS = N_FULL // NCORES  # 1024 per-core shard
P = 128
KO = K // P       # 16 k-tiles
MT = M // P       # 64 m-tiles


def build_nc():
    nc = bacc.Bacc("TRN2", target_bir_lowering=False, debug=False,
                   num_devices=NCORES)
    x_d = nc.dram_tensor("x", [M, K], BF16, kind="ExternalInput")
    q_d = nc.dram_tensor("q", [P, KO * NS], FP8, kind="ExternalInput")
    o_d = nc.dram_tensor("out", [M, NS], BF16, kind="ExternalOutput")
    x_ap, q_ap, o_ap = x_d.ap(), q_d.ap(), o_d.ap()

    with tile.TileContext(nc) as tc:
        with (
            tc.tile_pool(name="qpool", bufs=1) as qpool,
            tc.tile_pool(name="xpool", bufs=12) as xpool,
            tc.tile_pool(name="opool", bufs=4) as opool,
            tc.tile_pool(name="psum_o", bufs=8, space="PSUM") as psum_o,
        ):
            # ---- resident ternary weights (moving operand) ------------
            tile_q = qpool.tile([P, KO * NS], FP8, name="q")
            for ko in range(KO):
                eng = nc.sync if ko % 2 == 0 else nc.gpsimd
                eng.dma_start(tile_q[:, ko * NS:(ko + 1) * NS],
                              q_ap[:, ko * NS:(ko + 1) * NS])

            # ---- main loop: out[m, n] = sum_k x[m,k] q[n,k] -----------
            for mt in range(MT):
                xt = xpool.tile([P, K], BF16, name=f"x_{mt}", tag="x")
                nc.scalar.dma_start(xt[:], x_ap[mt * P:(mt + 1) * P, :])
                psA = psum_o.tile([P, 512], F32, name=f"psA_{mt}", tag="ps")
                psB = psum_o.tile([P, 512], F32, name=f"psB_{mt}", tag="ps")
                for ko in range(KO):
                    nc.tensor.matmul(
                        psA[:], lhsT=xt[:, ko * P:(ko + 1) * P],
                        rhs=tile_q[:, ko * NS:ko * NS + 512],
                        start=(ko == 0), stop=(ko == KO - 1))
                    nc.tensor.matmul(
                        psB[:], lhsT=xt[:, ko * P:(ko + 1) * P],
                        rhs=tile_q[:, ko * NS + 512:(ko + 1) * NS],
                        start=(ko == 0), stop=(ko == KO - 1))
                ot = opool.tile([P, NS], BF16, name=f"o_{mt}", tag="o")
                nc.vector.tensor_scalar(
                    ot[:, 0:512], psA[:], 1.0, None, mybir.AluOpType.mult)
                nc.vector.tensor_scalar(
                    ot[:, 512:1024], psB[:], 1.0, None, mybir.AluOpType.mult)
                nc.sync.dma_start(o_ap[mt * P:(mt + 1) * P, :], ot[:])

    nc.compile()
    return nc


_NC_CACHE = None


def get_nc():
    global _NC_CACHE
    if _NC_CACHE is None:
        _NC_CACHE = build_nc()
    return _NC_CACHE


def make_in_maps(x, weight):
    x2 = np.asarray(x, dtype=np.float32).reshape(M, K)
    w = np.asarray(weight, dtype=np.float32)

    # exact reference prep: scale from the full W, ternary q
    scale = np.float32(1e-4) + np.abs(w).mean(dtype=np.float32)
    q = np.clip(np.rint(w / scale), -1.0, 1.0).astype(np.float32)

    # xdev[mt*128+p, ko*128+m] = scale * x[mt*128+m, ko*128+p]
    xs = (x2 * scale).reshape(MT, P, KO, P)
    xdev = np.ascontiguousarray(
        xs.transpose(0, 3, 2, 1).reshape(M, K).astype(BF16_NP))

    # qdev_c[p, ko*1024+n] = q[c*1024+n, ko*128+p]  (ternary: exact in fp8)
    q4 = q.reshape(NCORES, NS, KO, P).transpose(0, 3, 2, 1)  # [c, p, ko, n]
    qdev = np.ascontiguousarray(q4.reshape(NCORES, P, KO * NS).astype(FP8_NP))

    return [{"x": xdev, "q": qdev[c]} for c in range(NCORES)]


def kernel(x, weight):
    nc = get_nc()
    in_maps = make_in_maps(x, weight)
    try:
        res = run_bass_kernel_spmd(nc, in_maps, list(range(NCORES)))
    except Exception:
        # transient device errors have been observed on first touch; retry once
        res = run_bass_kernel_spmd(nc, in_maps, list(range(NCORES)))
    out = np.concatenate(
        [np.asarray(res.results[c]["out"]) for c in range(NCORES)], axis=1)
    return np.ascontiguousarray(out, dtype=np.float32).reshape(4, 2048, N_FULL)


# revision 9
# speedup vs baseline: 1.1879x; 1.1879x over previous
"""BitNet linear layer (b1.58-style) on 8 Trainium2 NeuronCores.

Computes: scale = 1e-4 + mean(|W|); q = clip(round(W/scale), -1, 1);
          out = scale * (x @ q.T)
for x [4, 2048, 2048] f32 and W [8192, 2048] f32.

Sharding: tensor-parallel over out_features. Each core gets the full x
(replicated) and a 1024-row shard of the ternary q; cores run fully
independently and the host concatenates the per-core [8192, 1024]
output slices along the feature axis.

The elementwise prep is done once on the host (it is ~0.1% of the FLOPs
and would otherwise be redundantly recomputed per core): the exact
global scale and ternary q (bit-identical rounding vs the reference),
the f32->bf16 casts, and the transposes into SBUF-ready layouts.
`scale` is folded into the bf16 x cast, which is free in accuracy terms
(a single bf16 rounding either way), so the device applies no scale at
all. Remaining error is just the bf16 rounding of x (~2.3e-3).

The device is then a pure gap-free bf16 matmul at the PE roofline
(2048 matmuls of N=512 at ~216 ns cadence ~= 443 us):

  - xdev [8192, 2048] bf16 (replicated): row mt*128+p, col ko*128+m
    holds scale*x[token mt*128+m, k = ko*128+p] -- i.e. 64 m-tiles,
    each a [128k x (16ko x 128m)] stationary-operand block, 4 KiB
    contiguous per partition. One 512 KiB DMA per m-tile on the
    scalar queue, prefetched ~12 tiles deep.
  - qdev [128, 16384] bf16 (per-core shard): col ko*1024+n holds
    q[n-th row of shard, ko*128+p]. Loaded as 16 per-ko slices,
    even ko on the sync queue, odd ko on gpsimd, so the first
    matmul only waits for slice 0 and the rest land under compute.
  - Main loop over 64 m-tiles: x tile is the stationary operand
    (LDWEIGHTS hides under the moving stream), q is the moving
    operand; 16 k-steps of two 512-col accumulating matmuls into a
    psum bank pair (8 banks -> 4 m-tiles in flight). DVE drains
    psum -> f32 out tile; out rows stored in natural [M, N-shard]
    orientation on the sync queue.
"""

import sys

sys.path.insert(0, "/opt/trn_rl_repo")

import numpy as np
import ml_dtypes

import concourse.bass as bass
import concourse.tile as tile
from concourse import bacc, mybir
from concourse.bass_utils import run_bass_kernel_spmd

F32 = mybir.dt.float32
BF16 = mybir.dt.bfloat16
FP8 = mybir.dt.float8e4
BF16_NP = ml_dtypes.bfloat16
FP8_NP = ml_dtypes.float8_e4m3

NCORES = 8
M = 8192          # tokens (4*2048)
K = 2048          # in_features
N_FULL = 8192     # out_features
NS = N_FULL // NCORES  # 1024 per-core shard
P = 128
KO = K // P       # 16 k-tiles
MT = M // P       # 64 m-tiles


def build_nc():
    nc = bacc.Bacc("TRN2", target_bir_lowering=False, debug=False,
                   num_devices=NCORES)
    x_d = nc.dram_tensor("x", [M, K], BF16, kind="ExternalInput")
    qh_d = nc.dram_tensor("qh", [P, 2 * NS], BF16, kind="ExternalInput")
    qt_d = nc.dram_tensor("qt", [P, (KO - 2) * NS], FP8, kind="ExternalInput")
    o_d = nc.dram_tensor("out", [M, NS], BF16, kind="ExternalOutput")
    x_ap, qh_ap, qt_ap, o_ap = x_d.ap(), qh_d.ap(), qt_d.ap(), o_d.ap()

    with tile.TileContext(nc) as tc:
        with (
            tc.tile_pool(name="qpool", bufs=1) as qpool,
            tc.tile_pool(name="xpool", bufs=12) as xpool,
            tc.tile_pool(name="opool", bufs=4) as opool,
            tc.tile_pool(name="psum_o", bufs=8, space="PSUM") as psum_o,
        ):
            # ---- resident ternary weights (moving operand) ------------
            # ko 0-1 land first as plain bf16 on the sync queue; ko 2-15
            # ride the gpsimd queue as fp8 with cast-during-DMA, halving
            # their HBM bytes so the startup burst clears sooner. The
            # matmul always reads bf16 (an fp8 moving operand streams
            # ~20% slower: 259 vs 216 ns/MM measured).
            tile_q = qpool.tile([P, KO * NS], BF16, name="q")
            for ko in range(2):
                nc.sync.dma_start(tile_q[:, ko * NS:(ko + 1) * NS],
                                  qh_ap[:, ko * NS:(ko + 1) * NS])
            for g in range(7):
                ko = 2 + 2 * g
                nc.gpsimd.dma_start(
                    tile_q[:, ko * NS:(ko + 2) * NS],
                    qt_ap[:, (ko - 2) * NS:ko * NS])

            # ---- main loop: out[m, n] = sum_k x[m,k] q[n,k] -----------
            for mt in range(MT):
                xt = xpool.tile([P, K], BF16, name=f"x_{mt}", tag="x")
                if mt == 0:
                    # chunked so the first matmul only waits for 1/4 tile
                    for c in range(4):
                        nc.scalar.dma_start(
                            xt[:, c * 512:(c + 1) * 512],
                            x_ap[:P, c * 512:(c + 1) * 512])
                else:
                    nc.scalar.dma_start(xt[:], x_ap[mt * P:(mt + 1) * P, :])
                psA = psum_o.tile([P, 512], F32, name=f"psA_{mt}", tag="ps")
                psB = psum_o.tile([P, 512], F32, name=f"psB_{mt}", tag="ps")
                for ko in range(KO):
                    nc.tensor.matmul(
                        psA[:], lhsT=xt[:, ko * P:(ko + 1) * P],
                        rhs=tile_q[:, ko * NS:ko * NS + 512],
                        start=(ko == 0), stop=(ko == KO - 1))
                    nc.tensor.matmul(
                        psB[:], lhsT=xt[:, ko * P:(ko + 1) * P],
                        rhs=tile_q[:, ko * NS + 512:(ko + 1) * NS],
                        start=(ko == 0), stop=(ko == KO - 1))
                ot = opool.tile([P, NS], BF16, name=f"o_{mt}", tag="o")
                nc.vector.tensor_scalar(
                    ot[:, 0:512], psA[:], 1.0, None, mybir.AluOpType.mult)
                nc.vector.tensor_scalar(
                    ot[:, 512:1024], psB[:], 1.0, None, mybir.AluOpType.mult)
                nc.sync.dma_start(o_ap[mt * P:(mt + 1) * P, :], ot[:])

    nc.compile()
    return nc


_NC_CACHE = None


def get_nc():
    global _NC_CACHE
    if _NC_CACHE is None:
        _NC_CACHE = build_nc()
    return _NC_CACHE


def make_in_maps(x, weight):
    x2 = np.asarray(x, dtype=np.float32).reshape(M, K)
    w = np.asarray(weight, dtype=np.float32)

    # exact reference prep: scale from the full W, ternary q
    scale = np.float32(1e-4) + np.abs(w).mean(dtype=np.float32)
    q = np.clip(np.rint(w / scale), -1.0, 1.0).astype(np.float32)

    # xdev[mt*128+p, ko*128+m] = scale * x[mt*128+m, ko*128+p]
    xs = (x2 * scale).reshape(MT, P, KO, P)
    xdev = np.ascontiguousarray(
        xs.transpose(0, 3, 2, 1).reshape(M, K).astype(BF16_NP))

    # qdev_c[p, ko*1024+n] = q[c*1024+n, ko*128+p]  (ternary: exact in fp8)
    q4 = q.reshape(NCORES, NS, KO, P).transpose(0, 3, 2, 1)  # [c, p, ko, n]
    qdev = q4.reshape(NCORES, P, KO * NS)
    qh = np.ascontiguousarray(qdev[:, :, :2 * NS].astype(BF16_NP))
    qt = np.ascontiguousarray(qdev[:, :, 2 * NS:].astype(FP8_NP))

    return [{"x": xdev, "qh": qh[c], "qt": qt[c]} for c in range(NCORES)]


def kernel(x, weight):
    nc = get_nc()
    in_maps = make_in_maps(x, weight)
    try:
        res = run_bass_kernel_spmd(nc, in_maps, list(range(NCORES)))
    except Exception:
        # transient device errors have been observed on first touch; retry once
        res = run_bass_kernel_spmd(nc, in_maps, list(range(NCORES)))
    out = np.concatenate(
        [np.asarray(res.results[c]["out"]) for c in range(NCORES)], axis=1)
    return np.ascontiguousarray(out, dtype=np.float32).reshape(4, 2048, N_FULL)
